# revision 5
# baseline (speedup 1.0000x reference)
"""ActivateAttention Trainium2 kernel — 8 NeuronCores, SPMD, head-sharded.

Sharding: core i handles batch b=i//4 and head-group g=i%4 (3 of the 12
heads: ha=3g, hb=3g+1, hc=3g+2), all 4096 queries, full K/V for its
batch. Each core returns a PARTIAL output x_g @ Wp[:, 192g:192g+192].T;
the host sums the 4 group partials per batch and adds bp.

Per-core pipeline (fp16 compute, f32 PSUM accumulate):
  lead-in: weight slices DMA -> cast fp16 -> PE-transpose;
           K projected chunk-wise (x^T via PE transpose, W^T.T @ x^T,
           +bias, exact GELU on ACT) into kT pair/solo tiles;
           V projected per 128-row tile into v_aug (ones column per head);
           all interleaved with the first attention pass (Tile deps).
  attn:    8 passes over q-halves (512 cols). Per k-tile step the two
           paired heads' S^T [128,512] land in adjacent PSUM banks from
           two matmuls issued back-to-back into DIFFERENT PE row groups
           (partitions 0-63 / 64-127) so they stream concurrently.
           exp(SCALE*S): ACT handles cols [0:EXP_CUT) exactly; the DVE
           handles the rest via a one-instruction Schraudolph exp2
           (affine f32 -> int16 convert, bitcast as fp16). PV accumulates
           [65,512] per head (ones column -> softmax denominators).
           The solo head hc is row-paired across adjacent k-tiles using
           duplicated qT/kT copies in both partition halves.
  tail:    per finished q-half: x * recip(denominator) -> xT fp16;
           out_partial = xT.T @ Wp_slice^T (no bias - host adds bp).
"""

import numpy as np
from contextlib import ExitStack

from concourse import bass, bacc, mybir, masks, tile
from concourse import bass_utils

F32 = mybir.dt.float32
FP16 = mybir.dt.float16
I16 = mybir.dt.int16
AF = mybir.ActivationFunctionType
ALU = mybir.AluOpType

B = 2
N = 4096
DIM = 768
H = 12
D = 64
SCALE = D ** -0.5            # 1/8
N_CORES = 8
HG = 3                       # heads per core
GD = HG * D                  # 192 output dims per core

NT_K = N // 128              # 32 key row-tiles
NCT = DIM // 128             # 6 input-channel tiles
NQH = N // 512               # 8 query halves
LOG2E = 1.4426950408889634

# exp split: ACT does S^T cols [0:EXP_CUT), DVE-Schraudolph does the rest
EXP_CUT = 1024               # 1024 = pure ACT (exact); tune down to ~512
SCH_A = SCALE * LOG2E * 1024.0
SCH_B = 15.0 * 1024.0 - 46.0


def build_nc() -> bass.Bass:
    nc = bacc.Bacc("TRN2", target_bir_lowering=False, debug=False)

    query = nc.declare_dram_parameter("query", [N, DIM], F32, False).ap()
    key = nc.declare_dram_parameter("key", [N, DIM], F32, False).ap()
    value = nc.declare_dram_parameter("value", [N, DIM], F32, False).ap()
    Wq = nc.declare_dram_parameter("Wq", [GD, DIM], F32, False).ap()
    Wk = nc.declare_dram_parameter("Wk", [GD, DIM], F32, False).ap()
    bk = nc.declare_dram_parameter("bk", [GD], F32, False).ap()
    Wv = nc.declare_dram_parameter("Wv", [GD, DIM], F32, False).ap()
    Wp = nc.declare_dram_parameter("Wp", [DIM, GD], F32, False).ap()
    out = nc.declare_dram_parameter("out", [N, DIM], F32, True).ap()

    with tile.TileContext(nc) as tc, ExitStack() as ctx:
        # ---------------- persistent SBUF ----------------
        cpool = ctx.enter_context(tc.tile_pool(name="const", bufs=1))
        ident = cpool.tile([128, 128], FP16)
        masks.make_identity(nc, ident[:])
        ones16 = cpool.tile([1, D], FP16)
        nc.vector.memset(ones16[:], 1.0)

        # k-proj biases as per-partition columns
        bk_pair = cpool.tile([128, 1], F32)
        nc.sync.dma_start(out=bk_pair[:], in_=bk[0:128].rearrange("(p a) -> p a", a=1))
        bk_solo = cpool.tile([128, 1], F32)
        nc.sync.dma_start(out=bk_solo[0:64, :], in_=bk[128:192].rearrange("(p a) -> p a", a=1))
        nc.sync.dma_start(out=bk_solo[64:128, :], in_=bk[128:192].rearrange("(p a) -> p a", a=1))

        # transposed weights
        # wqk_t blocks: [0:128) Wq^T pair, [128:256) Wq^T solo dup,
        #               [256:384) Wk^T pair, [384:512) Wk^T solo dup
        wqk_t = cpool.tile([128, NCT, 512], FP16)
        wv_t = cpool.tile([128, NCT, GD], FP16)
        wpA = cpool.tile([128, DIM], FP16)     # Wp_s^T rows 0:128 (pair dims)
        wpB = cpool.tile([64, DIM], FP16)      # Wp_s^T rows 128:192 (solo dims)

        # projected tensors (per-chunk tiles for fine-grained deps)
        qT_P = [cpool.tile([128, 1024], FP16, name=f"qTP{j}", tag=f"qTP{j}")
                for j in range(4)]
        qT_S = [cpool.tile([128, 1024], FP16, name=f"qTS{j}", tag=f"qTS{j}")
                for j in range(4)]
        kT_P = [cpool.tile([128, 1024], FP16, name=f"kTP{j}", tag=f"kTP{j}")
                for j in range(4)]
        kT_S = [cpool.tile([128, 1024], FP16, name=f"kTS{j}", tag=f"kTS{j}")
                for j in range(4)]
        v_aug = [cpool.tile([128, HG * 65], FP16, name=f"va{t}", tag=f"va{t}")
                 for t in range(NT_K)]
        xTa = [cpool.tile([128, 512], FP16, name=f"xTa{q}", tag=f"xTa{q}")
               for q in range(NQH)]
        xTb = [cpool.tile([64, 512], FP16, name=f"xTb{q}", tag=f"xTb{q}")
               for q in range(NQH)]

        # ---------------- pools ----------------
        # PSUM: spool 2x[128,1024]f32 = 4 banks, apool 3x[65,512]f32 = 3,
        # rpool 1x2KB = 1  -> 8 banks
        spool = ctx.enter_context(tc.tile_pool(name="spool", bufs=2, space="PSUM"))
        apool = ctx.enter_context(tc.tile_pool(name="apool", bufs=3, space="PSUM"))
        rpool = ctx.enter_context(tc.tile_pool(name="rpool", bufs=1, space="PSUM"))
        ldpool = ctx.enter_context(tc.tile_pool(name="ldpool", bufs=3))
        cast_pool = ctx.enter_context(tc.tile_pool(name="cast", bufs=3))
        xt_pool = ctx.enter_context(tc.tile_pool(name="xt", bufs=2))
        pt_pool = ctx.enter_context(tc.tile_pool(name="pt", bufs=3))
        dpool = ctx.enter_context(tc.tile_pool(name="drain", bufs=2))
        opool = ctx.enter_context(tc.tile_pool(name="out", bufs=2))

        # ---------------- weight prep ----------------
        def load_cast(src_ap, rows):
            wf = ldpool.tile([rows, DIM], F32, tag="wf")
            nc.sync.dma_start(out=wf[:], in_=src_ap)
            wb = cast_pool.tile([rows, DIM], FP16, tag="wb")
            nc.vector.tensor_copy(wb[:], wf[:])
            return wb

        def transp_to(wb, rows, dst_slices):
            """transpose [rows, DIM] fp16 into dst column slices per c-tile."""
            for c in range(NCT):
                tp = rpool.tile([128, rows], FP16, tag="rp")
                nc.tensor.transpose(tp[:], wb[:, 128 * c:128 * (c + 1)], ident[:rows, :rows])
                for dst in dst_slices:
                    nc.vector.tensor_copy(dst(c), tp[:])

        for wsrc, col0 in ((Wq, 0), (Wk, 256)):
            wb = load_cast(wsrc[0:128, :], 128)
            transp_to(wb, 128, [lambda c, col0=col0: wqk_t[:, c, col0:col0 + 128]])
            wb = load_cast(wsrc[128:192, :], 64)
            transp_to(wb, 64, [
                lambda c, col0=col0: wqk_t[:, c, col0 + 128:col0 + 192],
                lambda c, col0=col0: wqk_t[:, c, col0 + 192:col0 + 256]])
        wb = load_cast(Wv[0:128, :], 128)
        transp_to(wb, 128, [lambda c: wv_t[:, c, 0:128]])
        wb = load_cast(Wv[128:192, :], 64)
        transp_to(wb, 64, [lambda c: wv_t[:, c, 128:192]])
        for r in range(NCT):
            wf = ldpool.tile([128, GD], F32, tag="wf")
            nc.sync.dma_start(out=wf[:], in_=Wp[128 * r:128 * (r + 1), :])
            wb = cast_pool.tile([128, GD], FP16, tag="wb")
            nc.vector.tensor_copy(wb[:], wf[:])
            tp = rpool.tile([128, 128], FP16, tag="rp")
            nc.tensor.transpose(tp[:], wb[:, 0:128], ident[:])
            nc.vector.tensor_copy(wpA[:, 128 * r:128 * (r + 1)], tp[:])
            tp2 = rpool.tile([64, 128], FP16, tag="rp")
            nc.tensor.transpose(tp2[:], wb[:, 128:192], ident[:])
            nc.vector.tensor_copy(wpB[:, 128 * r:128 * (r + 1)], tp2[:])

        # ---------------- input chunk -> x^T -> projections ----------------
        def emit_chunk_xt(src_ap, j):
            """DMA+cast+transpose rows [1024j,1024j+1024) -> xt [128,6,1024]."""
            xt = xt_pool.tile([128, NCT, 1024], FP16, tag="xt")
            for t in range(8):
                row0 = 1024 * j + 128 * t
                xf = ldpool.tile([128, DIM], F32, tag="xf")
                nc.sync.dma_start(out=xf[:], in_=src_ap[row0:row0 + 128, :])
                xb = cast_pool.tile([128, DIM], FP16, tag="xb")
                nc.vector.tensor_copy(xb[:], xf[:])
                tp = rpool.tile([128, NCT, 128], FP16, tag="rp")
                for c in range(NCT):
                    nc.tensor.transpose(tp[:, c, :], xb[:, 128 * c:128 * (c + 1)],
                                        ident[:])
                nc.vector.tensor_copy(xt[:, :, 128 * t:128 * (t + 1)], tp[:])
            return xt

        def emit_proj_block(xt, wcol0, dst, gelu, bias):
            pp = spool.tile([128, 1024], F32, tag="sp")
            for c in range(NCT):
                for h2 in range(2):
                    nc.tensor.matmul(
                        pp[:, 512 * h2:512 * (h2 + 1)],
                        wqk_t[:, c, wcol0:wcol0 + 128],
                        xt[:, c, 512 * h2:512 * (h2 + 1)],
                        start=(c == 0), stop=(c == NCT - 1))
            if gelu:
                nc.scalar.activation(dst, pp[:], AF.Gelu, bias=bias, scale=1.0)
            else:
                nc.scalar.copy(dst, pp[:])

        def emit_k_chunk(j):
            xt = emit_chunk_xt(key, j)
            emit_proj_block(xt, 256, kT_P[j][:], True, bk_pair[:, 0:1])
            emit_proj_block(xt, 384, kT_S[j][:], True, bk_solo[:, 0:1])

        def emit_q_chunk(j):
            xt = emit_chunk_xt(query, j)
            emit_proj_block(xt, 0, qT_P[j][:], False, None)
            emit_proj_block(xt, 128, qT_S[j][:], False, None)

        def emit_v_tile(t):
            vf = ldpool.tile([128, DIM], F32, tag="vf")
            nc.sync.dma_start(out=vf[:], in_=value[128 * t:128 * (t + 1), :])
            vb = cast_pool.tile([128, DIM], FP16, tag="vb")
            nc.vector.tensor_copy(vb[:], vf[:])
            vt = cast_pool.tile([128, NCT, 128], FP16, tag="vt")
            tpv = rpool.tile([128, NCT, 128], FP16, tag="rp")
            for c in range(NCT):
                nc.tensor.transpose(tpv[:, c, :], vb[:, 128 * c:128 * (c + 1)],
                                    ident[:])
            nc.vector.tensor_copy(vt[:], tpv[:])
            pv = rpool.tile([128, GD], F32, tag="rp")
            for c in range(NCT):
                nc.tensor.matmul(pv[:], vt[:, c, :], wv_t[:, c, :],
                                 start=(c == 0), stop=(c == NCT - 1))
            dst3 = v_aug[t][:].rearrange("p (h w) -> p h w", w=65)
            nc.vector.tensor_copy(dst3[:, :, 0:64],
                                  pv[:].rearrange("p (h w) -> p h w", w=64))
            nc.vector.memset(dst3[:, :, 64:65], 1.0)

        # ---------------- attention ----------------
        def emit_exp(slot, pt):
            if EXP_CUT > 0:
                nc.scalar.activation(pt[:, 0:EXP_CUT], slot[:, 0:EXP_CUT],
                                     AF.Exp, scale=SCALE)
            if EXP_CUT < 1024:
                nc.vector.tensor_scalar(
                    pt[:, EXP_CUT:1024].bitcast(I16),
                    slot[:, EXP_CUT:1024],
                    SCH_A, SCH_B, ALU.mult, ALU.add)

        def emit_pair_step(qh, kt, xps_a, xps_b):
            q0 = 512 * (qh % 2)
            jq, jk = qh // 2, kt // 8
            c0 = 128 * (kt % 8)
            slot = spool.tile([128, 1024], F32, tag="sp", name=f"sP{qh}_{kt}")
            nc.tensor.matmul(slot[:, 0:512], kT_P[jk][0:64, c0:c0 + 128],
                             qT_P[jq][0:64, q0:q0 + 512], start=True, stop=True)
            nc.tensor.matmul(slot[:, 512:1024], kT_P[jk][64:128, c0:c0 + 128],
                             qT_P[jq][64:128, q0:q0 + 512], start=True, stop=True)
            pt = pt_pool.tile([128, 1024], FP16, tag="pt", name=f"pP{qh}_{kt}")
            emit_exp(slot, pt)
            va = v_aug[kt][:]
            nc.tensor.matmul(xps_a[:], va[:, 0:65], pt[:, 0:512],
                             start=(kt == 0), stop=(kt == NT_K - 1),
                             skip_group_check=True)
            nc.tensor.matmul(xps_b[:], va[:, 65:130], pt[:, 512:1024],
                             start=(kt == 0), stop=(kt == NT_K - 1),
                             skip_group_check=True)

        def emit_solo_step(qh, ktp, xps_c):
            q0 = 512 * (qh % 2)
            jq = qh // 2
            kta, ktb = 2 * ktp, 2 * ktp + 1
            slot = spool.tile([128, 1024], F32, tag="sp", name=f"sS{qh}_{ktp}")
            nc.tensor.matmul(slot[:, 0:512],
                             kT_S[kta // 8][0:64, 128 * (kta % 8):128 * (kta % 8) + 128],
                             qT_S[jq][0:64, q0:q0 + 512], start=True, stop=True)
            nc.tensor.matmul(slot[:, 512:1024],
                             kT_S[ktb // 8][64:128, 128 * (ktb % 8):128 * (ktb % 8) + 128],
                             qT_S[jq][64:128, q0:q0 + 512], start=True, stop=True)
            pt = pt_pool.tile([128, 1024], FP16, tag="pt", name=f"pS{qh}_{ktp}")
            emit_exp(slot, pt)
            nc.tensor.matmul(xps_c[:], v_aug[kta][:, 130:195], pt[:, 0:512],
                             start=(ktp == 0), stop=False, skip_group_check=True)
            nc.tensor.matmul(xps_c[:], v_aug[ktb][:, 130:195], pt[:, 512:1024],
                             start=False, stop=(ktp == NT_K // 2 - 1),
                             skip_group_check=True)

        def emit_drain(xps, dst):
            """normalize xps [65,512] -> dst [64,512] fp16."""
            d16 = dpool.tile([1, 512], FP16, tag="d16")
            nc.vector.tensor_copy(d16[:], xps[64:65, :])
            Rp = rpool.tile([D, 512], F32, tag="rp")
            nc.tensor.matmul(Rp[:], ones16[:], d16[:], start=True, stop=True)
            Rs = dpool.tile([D, 512], F32, tag="Rs")
            nc.vector.reciprocal_approx_fast(Rs[:], Rp[:])
            nc.vector.tensor_tensor(dst, xps[0:64, :], Rs[:], op=ALU.mult)

        def emit_out_proj(qh):
            """out rows [512qh, 512qh+512): 4 q-tiles of 128."""
            for tq in range(4):
                po = spool.tile([128, DIM], F32, tag="sp", name=f"po{qh}_{tq}")
                for o0, w in ((0, 512), (512, 256)):
                    nc.tensor.matmul(po[:, o0:o0 + w],
                                     xTa[qh][:, 128 * tq:128 * (tq + 1)],
                                     wpA[:, o0:o0 + w], start=True, stop=False)
                    nc.tensor.matmul(po[:, o0:o0 + w],
                                     xTb[qh][:, 128 * tq:128 * (tq + 1)],
                                     wpB[:, o0:o0 + w], start=False, stop=True)
                ot = opool.tile([128, DIM], F32, tag="ot")
                nc.scalar.copy(ot[:], po[:])
                nc.sync.dma_start(
                    out=out[512 * qh + 128 * tq:512 * qh + 128 * (tq + 1), :],
                    in_=ot[:])

        emit_q_chunk(0)
        emit_k_chunk(0)
        for qh in range(NQH):
            xps_a = apool.tile([65, 512], F32, tag="ap", name=f"xa{qh}")
            xps_b = apool.tile([65, 512], F32, tag="ap", name=f"xb{qh}")
            xps_c = apool.tile([65, 512], F32, tag="ap", name=f"xc{qh}")
            for kt in range(NT_K):
                if qh == 0:
                    if kt % 8 == 2 and kt // 8 < 3:
                        emit_k_chunk(kt // 8 + 1)
                    emit_v_tile(kt)
                if qh in (1, 3, 5) and kt == 4:
                    emit_q_chunk((qh + 1) // 2)
                emit_pair_step(qh, kt, xps_a, xps_b)
                if kt % 2 == 1:
                    emit_solo_step(qh, kt // 2, xps_c)
            emit_drain(xps_a[:], xTa[qh][0:64, :])
            emit_drain(xps_b[:], xTa[qh][64:128, :])
            emit_drain(xps_c[:], xTb[qh][0:64, :])
            if qh >= 1:
                emit_out_proj(qh - 1)
        emit_out_proj(NQH - 1)

    nc.compile()
    return nc


_NC_CACHE = {}


def _get_nc():
    if "nc" not in _NC_CACHE:
        _NC_CACHE["nc"] = build_nc()
    return _NC_CACHE["nc"]


def make_in_maps(query, key, value, Wq, Wk, bk, Wv, Wp):
    in_maps = []
    for i in range(N_CORES):
        b, g = i // 4, i % 4
        gs = slice(GD * g, GD * (g + 1))
        in_maps.append({
            "query": query[b], "key": key[b], "value": value[b],
            "Wq": Wq[gs, :], "Wk": Wk[gs, :], "bk": bk[gs],
            "Wv": Wv[gs, :], "Wp": Wp[:, gs],
        })
    return [{k: np.ascontiguousarray(v, dtype=np.float32)
             for k, v in m.items()} for m in in_maps]


def reduce_out(res, bp):
    out = np.empty((B, N, DIM), dtype=np.float32)
    for b in range(B):
        acc = res.results[4 * b]["out"].astype(np.float32).copy()
        for g in range(1, 4):
            acc += res.results[4 * b + g]["out"]
        out[b] = acc + bp
    return out


def kernel(query, key, value, Wq, Wk, bk, Wv, Wp, bp, _results_hook=None):
    args = [np.asarray(a, dtype=np.float32)
            for a in (query, key, value, Wq, Wk, bk, Wv, Wp)]
    nc = _get_nc()
    in_maps = make_in_maps(*args)
    res = bass_utils.run_bass_kernel_spmd(nc, in_maps,
                                          core_ids=list(range(N_CORES)))
    if _results_hook is not None:
        _results_hook(res)
    return reduce_out(res, np.asarray(bp, dtype=np.float32))
